# revision 17
# baseline (speedup 1.0000x reference)
"""Trainium2 Bass kernel for nn_Actions_block_14388140442036 (gnn_message_passing).

The reference network is entirely linear (no activations), so the output
    out = segment_sum(actions) @ pol_W + pol_b
collapses to per-effect scalars:
    p[j] = actions[j] @ pol_W  (a dot product against fused weight vectors)
followed by a scalar segment-sum.  Folding pol_W through each branch:

  glob branch:  p_g[i] = (globs @ w1)[U[i]]     + action_globs[i]. w2 + cg
  node branch:  p_n[i] = (nodes @ w3)[V[i]]     + action_nodes[i]. w4 + cn
  edge branch:  p_e[i] = (edges[E[i]] . u1) + (nodes @ wr)[row[E[i]]]
                        + (nodes @ wc)[col[E[i]]] + action_edges[i]. wv + ce

where  w1|w2 = glob_W @ pol_W,  w3|w4 = node_W @ pol_W,
       u1|u2 = e2_W @ pol_W,    wr|wv|wc = e1_W @ u2.

Only ~22% of edge rows are ever referenced (E draws 100k effects from 400k
edges, ~88.5k distinct), so the referenced edge rows are gathered and
DEDUPLICATED on the host (per the sharding hint: data-parallel over action
effects with gathered features) and each distinct row streams through the
device once; the host maps the resulting dots back through the inverse
index.  Likewise only ~95% of nodes are touched by V/row[E]/col[E], so just
the referenced node rows stream, each carrying three fused weight vectors.

The device work is pure memory streaming (target_regime=memory): every
feature row contributes only 1-3 dot products, so wall-clock is the DMA-in
byte count.  All feature streams are pre-transposed to feature-major
[feat, row] fp8e3 (e3m4) on the host — measured end-to-end output error of
the fp8-data / fp16-weight pipeline is ~1.1e-2, comfortably inside the 2e-2
gate, and it halves the stream relative to fp16.  Each 128-row group is a
single PE matmul with the fp8 data tile as the stationary operand and the
1-3 fp16 fused weight columns as the moving operand — no on-device
transposes, no DVE reductions, fp32 PSUM accumulation.  Dots drain to SBUF
as fp16 via the ACT engine and DMA out in pieces so only the last small
piece's drain chain trails the final input transfer.  The host does the
tiny fused-weight precompute, the scalar gathers and the segment sum.
"""

import numpy as np
import ml_dtypes

import concourse.bacc as bacc
import concourse.mybir as mybir
import concourse.tile as tile
from concourse.bass_utils import run_bass_kernel_spmd

# ---- problem constants (hardcoded; kernel.py must be self-contained) ----
HID = 128
FEAT = 16
N_NODES = 100000
N_EDGES = 400000
N_PER = 100000
A_TOTAL = 300000
NUM_ACTIONS = 75000
N_CORES = 8

N_SH = N_NODES // N_CORES   # 12500 node rows per core
A_SH = N_PER // N_CORES     # 12500 action-effect rows per core (all branches)

G_A = 98                    # apack groups (12544 padded effect rows)
G_E = 88                    # deduplicated gathered-edge groups per core
G_N = 93                    # referenced-node groups per core
R_A = G_A * 128
R_E = G_E * 128             # 11264 unique edge rows per core (99.99%+ capacity)
R_N = G_N * 128             # 11904 referenced node rows per core
G1 = 48                     # nodes piece 1
G2 = 89                     # nodes piece 2 ends here; tail = 4 groups

F8 = mybir.dt.float8e3      # e3m4: 4 mantissa bits, max 15.5
F16 = mybir.dt.float16
F32 = mybir.dt.float32

_CACHE = {}


def _build_program(repeat=1):
    nc = bacc.Bacc("TRN2", target_bir_lowering=False, debug=False,
                   num_devices=N_CORES)

    egT_in = nc.dram_tensor("egT_in", [128, R_E], F8, kind="ExternalInput").ap()
    nodesT_in = nc.dram_tensor("nodesT_in", [128, R_N], F8, kind="ExternalInput").ap()
    apackT_in = nc.dram_tensor("apackT_in", [48, R_A], F8, kind="ExternalInput").ap()
    wts_in = nc.dram_tensor("wts_in", [128, 8], F16, kind="ExternalInput").ap()

    qe_out = nc.dram_tensor("qe_out", [128, G_E], F16, kind="ExternalOutput").ap()
    qn_out = nc.dram_tensor("qn_out", [128, 3 * G_N], F16, kind="ExternalOutput").ap()
    pa_out = nc.dram_tensor("pa_out", [128, 3 * G_A], F16, kind="ExternalOutput").ap()

    with tile.TileContext(nc) as tc:
        with (
            tc.tile_pool(name="wpool", bufs=1) as wpool,
            tc.tile_pool(name="dpool", bufs=1) as dpool,
            tc.tile_pool(name="opool", bufs=1) as opool,
            tc.tile_pool(name="pspool", bufs=1, space="PSUM") as pspool,
        ):
            wt = wpool.tile([128, 8], F16)
            nc.gpsimd.dma_start(wt[:], wts_in[:])

            ed = dpool.tile([128, R_E], F8, tag="ed")
            ad = dpool.tile([128, R_A], F8, tag="ad")  # partitions 0..47 used
            nd = dpool.tile([128, R_N], F8, tag="nd")

            ps_e = pspool.tile([128, G_E], F32, tag="pse")
            ps_a = pspool.tile([128, 3 * G_A], F32, tag="psa")
            ps_n = pspool.tile([128, 3 * G_N], F32, tag="psn")

            sb_e = opool.tile([128, G_E], F16, tag="sbe")
            sb_a = opool.tile([128, 3 * G_A], F16, tag="sba")
            sb_n = opool.tile([128, 3 * G_N], F16, tag="sbn")

            for _rep in range(repeat):
                # Input pieces in DMA order; nodes last, split so its dot /
                # drain chains overlap later input transfers, ending on a
                # small 4-group piece whose drain is the only tail.
                nc.sync.dma_start(ed[:, :], egT_in[:, :])
                nc.sync.dma_start(ad[0:48, :], apackT_in[:, :])
                nc.sync.dma_start(nd[:, 0:G1 * 128], nodesT_in[:, 0:G1 * 128])
                nc.sync.dma_start(nd[:, G1 * 128:G2 * 128],
                                  nodesT_in[:, G1 * 128:G2 * 128])
                nc.sync.dma_start(nd[:, G2 * 128:R_N], nodesT_in[:, G2 * 128:R_N])

                # Per 128-row group: one matmul, fp8 data stationary
                # [feat, rows], fp16 weight columns moving.
                for g in range(G_E):
                    nc.tensor.matmul(ps_e[:, g:g + 1],
                                     ed[:, g * 128:(g + 1) * 128],
                                     wt[:, 3:4])
                nc.scalar.copy(sb_e[:], ps_e[:])
                nc.sync.dma_start(qe_out[:], sb_e[:])

                for g in range(G_A):
                    nc.tensor.matmul(ps_a[:, g * 3:(g + 1) * 3],
                                     ad[0:48, g * 128:(g + 1) * 128],
                                     wt[0:48, 4:7])
                nc.scalar.copy(sb_a[:], ps_a[:])
                nc.sync.dma_start(pa_out[:], sb_a[:])

                for lo, hi, eng in ((0, G1, "sync"), (G1, G2, "sync"),
                                    (G2, G_N, "scalar")):
                    for g in range(lo, hi):
                        nc.tensor.matmul(ps_n[:, g * 3:(g + 1) * 3],
                                         nd[:, g * 128:(g + 1) * 128],
                                         wt[:, 0:3])
                    nc.scalar.copy(sb_n[:, 3 * lo:3 * hi], ps_n[:, 3 * lo:3 * hi])
                    getattr(nc, eng).dma_start(qn_out[:, 3 * lo:3 * hi],
                                               sb_n[:, 3 * lo:3 * hi])

    nc.compile()
    return nc


def _get_program():
    if "nc" not in _CACHE:
        _CACHE["nc"] = _build_program()
    return _CACHE["nc"]


def _unscramble_1(arr):
    """[128, g] -> [128*g]; row r = g*128 + p lives at arr[p, g]."""
    return np.ascontiguousarray(arr.T).reshape(-1).astype(np.float64)


def _unscramble_3(arr, groups):
    """[128, 3g] -> [128*g, 3]; row r = g*128 + p lives at arr[p, 3g:3g+3]."""
    return (arr.reshape(128, groups, 3).transpose(1, 0, 2)
            .reshape(-1, 3).astype(np.float64))


def _packT8(rows_f32, parts, cols):
    """[n, parts] f32 -> feature-major [parts, cols] fp8e3 (zero pad)."""
    out = np.zeros((parts, cols), ml_dtypes.float8_e3m4)
    q = np.clip(rows_f32.T, -15.5, 15.5).astype(ml_dtypes.float8_e3m4)
    out[:, :rows_f32.shape[0]] = q
    return out


def kernel(**inputs):
    inputs = {k: np.asarray(v) for k, v in inputs.items()}
    globs = inputs["globs"]
    nodes = np.ascontiguousarray(inputs["nodes"])
    edges = np.ascontiguousarray(inputs["edges"])
    action_globs = inputs["action_globs"]
    action_nodes = inputs["action_nodes"]
    action_edges = inputs["action_edges"]
    glob_W = inputs["glob_W"]; glob_b = inputs["glob_b"]
    node_W = inputs["node_W"]; node_b = inputs["node_b"]
    e1_W = inputs["e1_W"]; e1_b = inputs["e1_b"]
    e2_W = inputs["e2_W"]; e2_b = inputs["e2_b"]
    pol_W = inputs["pol_W"]; pol_b = inputs["pol_b"]
    row = inputs["row"]; col = inputs["col"]
    U = inputs["U"]; UA = inputs["UA"]; V = inputs["V"]; VA = inputs["VA"]
    E = inputs["E"]; EA = inputs["EA"]
    actions_batch = inputs["actions_batch"]

    # ---- fused weight vectors (float64 for accuracy; cast to f16 on device) ----
    polW = pol_W.astype(np.float64)[:, 0]                 # [128]
    g_f = glob_W.astype(np.float64) @ polW                # [144]
    n_f = node_W.astype(np.float64) @ polW                # [144]
    e2_f = e2_W.astype(np.float64) @ polW                 # [256]
    u1, u2 = e2_f[:HID], e2_f[HID:]
    e1_f = e1_W.astype(np.float64) @ u2                   # [272]
    w1, w2 = g_f[:HID], g_f[HID:]
    w3, w4 = n_f[:HID], n_f[HID:]
    wr, wv, wc = e1_f[:HID], e1_f[HID:HID + FEAT], e1_f[HID + FEAT:]
    cg = float(glob_b.astype(np.float64) @ polW)
    cn = float(node_b.astype(np.float64) @ polW)
    ce = float(e2_b.astype(np.float64) @ polW + e1_b.astype(np.float64) @ u2)

    # wts [128, 8]: cols 0-2 = w3|wr|wc, col 3 = u1,
    # cols 4-6 = block-diag [w2;w4;wv] over packed-feature partitions 0..47.
    wts = np.zeros((128, 8), np.float16)
    wts[:, 0] = w3.astype(np.float16)
    wts[:, 1] = wr.astype(np.float16)
    wts[:, 2] = wc.astype(np.float16)
    wts[:, 3] = u1.astype(np.float16)
    wts[0:FEAT, 4] = w2.astype(np.float16)
    wts[FEAT:2 * FEAT, 5] = w4.astype(np.float16)
    wts[2 * FEAT:3 * FEAT, 6] = wv.astype(np.float16)

    # gathered edge features, deduplicated: E references ~88.5k distinct of
    # 400k edge rows; ship each referenced row once and gather the dots by
    # the inverse map on the host.
    e_uniq, e_inv = np.unique(E, return_inverse=True)
    # referenced nodes only (~95%): V/row[E]/col[E] never touch the rest
    n_ref = np.unique(np.concatenate([V, row[E], col[E]]))
    if len(e_uniq) > N_CORES * R_E or len(n_ref) > N_CORES * R_N:
        # astronomically unlikely under the spec's distributions; fall back
        # to no dedup (identity maps padded into the full capacity would
        # overflow, so just error loudly rather than emit wrong results)
        raise RuntimeError("dedup capacity exceeded: "
                           f"{len(e_uniq)} edge rows, {len(n_ref)} node rows")
    eg = edges[e_uniq]                                     # [<=90112, 128]
    nodes_ref = nodes[n_ref]                               # [<=95232, 128]
    n_pos = np.zeros(N_NODES, np.int64)
    n_pos[n_ref] = np.arange(len(n_ref))

    # packed action features [N_PER, 48] = [ag | an | ae]
    apack = np.empty((N_PER, 3 * FEAT), np.float32)
    apack[:, :FEAT] = action_globs
    apack[:, FEAT:2 * FEAT] = action_nodes
    apack[:, 2 * FEAT:] = action_edges

    nc = _get_program()
    in_maps = []
    for c in range(N_CORES):
        in_maps.append({
            "egT_in": _packT8(eg[c * R_E:(c + 1) * R_E], 128, R_E),
            "nodesT_in": _packT8(nodes_ref[c * R_N:(c + 1) * R_N], 128, R_N),
            "apackT_in": _packT8(apack[c * A_SH:(c + 1) * A_SH], 48, R_A),
            "wts_in": wts,
        })
    res = run_bass_kernel_spmd(nc, in_maps, core_ids=list(range(N_CORES)))

    qe_u = np.empty(N_CORES * R_E, np.float64)            # unique-edge dots
    qn3 = np.empty((N_CORES * R_N, 3), np.float64)        # referenced-node dots
    pa = np.empty((N_PER, 3), np.float64)
    for c in range(N_CORES):
        r = res.results[c]
        qe_u[c * R_E:(c + 1) * R_E] = _unscramble_1(r["qe_out"])
        qn3[c * R_N:(c + 1) * R_N] = _unscramble_3(r["qn_out"], G_N)
        pa[c * A_SH:(c + 1) * A_SH] = _unscramble_3(r["pa_out"], G_A)[:A_SH]

    # ---- host: gathers, scatter into action slots, segment sum ----
    qg = globs.astype(np.float64) @ w1                    # [512]
    p_g = qg[U] + pa[:, 0] + cg
    p_n = qn3[n_pos[V], 0] + pa[:, 1] + cn
    p_e = (qe_u[e_inv] + qn3[n_pos[row[E]], 1] + qn3[n_pos[col[E]], 2]
           + pa[:, 2] + ce)

    actions_p = np.zeros(A_TOTAL, np.float64)
    actions_p[UA] = p_g
    actions_p[VA] = p_n
    actions_p[EA] = p_e

    # torch-style _norm: consecutive group ids starting at actions_batch[0]
    ab = actions_batch.astype(np.int64)
    changed = ab[1:] != ab[:-1]
    seg = int(ab[0]) + np.concatenate([[0], np.cumsum(changed)])
    if seg[0] >= 0 and seg[-1] < NUM_ACTIONS:
        agg = np.bincount(seg, weights=actions_p, minlength=NUM_ACTIONS)[:NUM_ACTIONS]
    else:  # jax segment_sum drops out-of-range ids
        agg = np.zeros(NUM_ACTIONS, np.float64)
        valid = (seg >= 0) & (seg < NUM_ACTIONS)
        np.add.at(agg, seg[valid], actions_p[valid])

    out = agg + float(pol_b.astype(np.float64)[0])
    return out.astype(np.float32)[:, None]
